# revision 44
# baseline (speedup 1.0000x reference)
"""Trainium2 Bass kernel for nn_Encoder_55293408969294 — v3: 8-way time shard.

Why v3 beats v2 (334 us):
  - The wall is the per-step serial chain  h_t -> W_hh matmuls -> sigmoid ->
    cell -> tanh -> h_{t+1}  (~4.2 us in v2), which no amount of stream
    interleaving can pipeline away: step t+1 of a stream cannot start before
    its own step t finishes.  Total time ~= NS * chain.  v3 therefore cuts NS:
    8 time segments (one per core, FULL batch 512 per core) with an L=8-step
    zero-state warmup, NS = (256+7*8)/8 = 39 steps (vs 70), and instead makes
    each step WIDER (2 batch pairs of 256) so the engines stay busy inside the
    longer-but-fewer chains.
  - h is carried in fp8-e4m3 and the h-side matmuls use DoubleRow perf mode
    (fp8, 2 k-tiles of 128 = the two hidden halves in one 0.5-cycle/row
    matmul).  Weight quantization noise is killed by a two-term expansion
    W_hh ~ fp8(W) + fp8(W - fp8(W)): two DoubleRow matmuls at half cost each,
    ~fp16 weight fidelity (1.2e-2 end-to-end vs 1.5e-2 single-term; h-fp8
    itself adds ~8e-3 over the 5e-3 numpy baseline; budget 2e-2).
  - All 8 gate slots get ONE sigmoid ACT call per pair ([128,2048]; ACT has a
    185ns fixed access overhead per call) using tanh(z)=2*sig(2z)-1 with the
    2x folded into host-scaled weights; tanh(c) is one sig(2c) call.
  - The per-gate bias enters PSUM via a single DoubleRow matmul from an
    8-partition one-hot (out free 2048 at 0.2 ns/row), not 2 fp16 matmuls.
  - DVE keeps only the latency-critical cell chain (v, t1, cn, h8); the
    off-chain products (xh = a*x for step u+1, and the fp16 output copy
    y = h') run on the otherwise-idle GPSIMD/Pool engine.
  - Attention (softmax over drives of e_x = x . w_x, constant over t) is
    batch-sharded: each core reduces its OWN 64 batch rows over the full
    window from a [t,d,b] fp8 transpose (2.1 MB, not 8.4), then an AllGather
    collective (64 KB) replicates a to all cores.
"""

import os
import numpy as np
import ml_dtypes  # noqa: F401

B, D, W, H = 512, 128, 256, 256
NCORES = 8

T_SHARD = 8
LWARM = int(os.environ.get("ENC_L", "8"))
NS = (W + (T_SHARD - 1) * LWARM) // T_SHARD   # 39 local steps per core
SEGV = NS - LWARM                             # valid steps, segments >= 1
BC = B                                        # full batch per core
WS = BC // 2                                  # pair width (256)
BSH = B // NCORES                             # attention batch shard (64)
TCH = int(os.environ.get("ENC_TCH", "1"))
WRES = os.environ.get("ENC_WRES", "1") == "1"

# slot order (i0,i1,g0,g1,f0,f1,o0,o1): the {i,g} half feeds v (the head of
# the cell chain) from its own PSUM tile; {f,o} matmuls+sigmoid overlap it.
PERM = [0, 1, 4, 5, 2, 3, 6, 7]   # slot -> original (i,f,g,o) block

_CACHE = {}
LAST_EXEC_NS = None


def _build_program(*_ignored):
    import concourse.bacc as bacc
    import concourse.bass as bass  # noqa: F401
    import concourse.mybir as mybir
    import concourse.tile as tile
    from concourse.masks import make_identity
    from contextlib import ExitStack

    f32 = mybir.dt.float32
    f16 = mybir.dt.float16
    f8 = mybir.dt.float8e4
    AF = mybir.ActivationFunctionType
    OP = mybir.AluOpType
    PM = mybir.MatmulPerfMode

    nc = bacc.Bacc("TRN2", target_bir_lowering=False, debug=False)

    xs_d = nc.dram_tensor("xseg", [D, NS, BC], f8, kind="ExternalInput")
    xt_d = nc.dram_tensor("xt8", [2, 4, 128, D, BSH // 4], f8,
                          kind="ExternalInput")
    wxt_d = nc.dram_tensor("wxt", [128, 2], f8, kind="ExternalInput")
    wih_d = nc.dram_tensor("wih", [D, 8 * 128], f16, kind="ExternalInput")
    whh8_d = nc.dram_tensor("whh8", [128, 2, 8, 128], f8, kind="ExternalInput")
    whhd_d = nc.dram_tensor("whhd", [128, 2, 8, 128], f8, kind="ExternalInput")
    b8_d = nc.dram_tensor("b8dr", [8, 128], f16, kind="ExternalInput")
    e8_d = nc.dram_tensor("e8dr", [8, 8 * WS], f16, kind="ExternalInput")
    # out: y[p, u*2+ht, b] = h_u[ht*128+p, b]
    y_d = nc.dram_tensor("y", [128, NS * 2, BC], f16, kind="ExternalOutput")

    with tile.TileContext(nc) as tc:
        with ExitStack() as ctx:
            singles = ctx.enter_context(tc.tile_pool(name="singles", bufs=1))
            psum_g = ctx.enter_context(
                tc.tile_pool(name="pg", bufs=1, space="PSUM"))
            dram = ctx.enter_context(
                tc.tile_pool(name="dram", bufs=1, space="DRAM"))
            xh_pool = ctx.enter_context(tc.tile_pool(name="xhp", bufs=3))
            sp_pool = ctx.enter_context(tc.tile_pool(name="spp", bufs=2))
            tmp_pool = ctx.enter_context(tc.tile_pool(name="tmpp", bufs=2))
            st_pool = ctx.enter_context(tc.tile_pool(name="stp", bufs=2))
            out_pool = ctx.enter_context(tc.tile_pool(name="outp", bufs=3))

            xs_sb = singles.tile([D, NS, BC], f8, name="xs_sb")
            xt_sb = singles.tile([128, 2, 4, D, BSH // 4], f8, name="xt_sb")
            wxt_sb = singles.tile([128, 2], f8, name="wxt_sb")
            wih_sb = singles.tile([128, 8 * 128], f16, name="wih_sb")
            whh8_sb = singles.tile([128, 2, 8, 128], f8, name="whh8_sb")
            whhd_sb = singles.tile([128, 2, 8, 128], f8, name="whhd_sb")
            b8_sb = singles.tile([8, 128], f16, name="b8_sb")
            e8_sb = singles.tile([8, 8 * WS], f16, name="e8_sb")
            id_sb = singles.tile([128, 128], f32, name="id_sb")
            exT = singles.tile([128, BSH], f32, name="exT")
            sg_sb = singles.tile([BSH, 128], f32, name="sg_sb")
            den_sb = singles.tile([BSH, 128], f32, name="den_sb")
            num_sb = singles.tile([BSH, 128], f32, name="num_sb")
            ssum = singles.tile([BSH, 1], f32, name="ssum")
            rr = singles.tile([BSH, 1], f32, name="rr")
            ab_sb = singles.tile([BSH, 128], f32, name="ab_sb")
            a8_sb = singles.tile([128, BSH], f8, name="a8_sb")
            aT_sb = singles.tile([128, NCORES, BSH], f8, name="aT_sb")

            # DMA priority: wxt + the attention transpose feed the collective
            # that gates the whole recurrence; weights follow; xseg last.
            nc.sync.dma_start(out=wxt_sb, in_=wxt_d.ap())
            make_identity(nc, id_sb)
            # b-chunk-major host layout: every chunk is contiguous on both
            # sides, so the attention matmuls pipeline under the transfer
            xtr = xt_d.ap().rearrange("tt bq tp d b -> tp tt bq d b")
            BQ = BSH // 4
            for tt in range(2):
                nc.sync.dma_start(
                    out=xt_sb[:, tt], in_=xtr[:, tt])
            nc.sync.dma_start(out=wih_sb, in_=wih_d.ap())
            nc.sync.dma_start(out=whh8_sb, in_=whh8_d.ap())
            if WRES:
                nc.sync.dma_start(out=whhd_sb, in_=whhd_d.ap())
            nc.sync.dma_start(out=b8_sb, in_=b8_d.ap())
            nc.sync.dma_start(out=e8_sb, in_=e8_d.ap())
            # only the first xseg chunk is loaded up front: the rest would
            # queue ahead of the tiny all-gather bounce DMA on the serialized
            # DMA engines and delay the collective; they stream in during the
            # first loop steps instead (chunk k is consumed from step ~10k).
            XCH = (NS + 3) // 4

            def load_xseg(tk, eng=None):
                lo = tk * XCH
                hi = min(lo + XCH, NS)
                if lo < hi:
                    (eng or nc.sync).dma_start(
                        out=xs_sb[:, lo:hi, :], in_=xs_d.ap()[:, lo:hi, :])

            load_xseg(0)

            # ---- attention (own 64 batch rows over the full window) ----
            gA = psum_g.tile([128, 4, WS], f32, tag="gigA", name="gA_att")
            gB = psum_g.tile([128, 4, WS], f32, tag="gigB", name="gB_att")
            gC = psum_g.tile([128, 4, WS], f32, tag="gfoA", name="gC_att")
            e_ps = gA.rearrange("p s w -> p (s w)")[:, 0:BSH]
            eb_ps = gB[0:BSH].rearrange("p s w -> p (s w)")[:, 0:128]
            for tt in range(2):
                for b in range(BSH):
                    nc.tensor.matmul(
                        e_ps[:, b:b + 1],
                        xt_sb[:, tt, b // BQ, :, b % BQ],
                        wxt_sb[:, tt:tt + 1],
                        start=(tt == 0), stop=(tt == 1))
            nc.vector.tensor_copy(exT, e_ps)
            nc.tensor.transpose(eb_ps, exT, id_sb)
            # exp(z) = sig(z)/(1-sig(z)) keeps everything on the Sigmoid table
            nc.scalar.activation(sg_sb, eb_ps, AF.Sigmoid)
            nc.vector.tensor_scalar(
                out=den_sb, in0=sg_sb, scalar1=-1.0, scalar2=1.0,
                op0=OP.mult, op1=OP.add)
            nc.vector.reciprocal(den_sb, den_sb)
            nc.vector.scalar_tensor_tensor(
                out=num_sb, in0=sg_sb, scalar=1.0, in1=den_sb,
                op0=OP.mult, op1=OP.mult, accum_out=ssum)
            nc.vector.reciprocal(rr, ssum)
            nc.vector.tensor_scalar_mul(ab_sb, num_sb, rr)
            a_ps = gC.rearrange("p s w -> p (s w)")[:, 0:BSH]
            nc.tensor.transpose(a_ps, ab_sb, id_sb[0:BSH, 0:BSH])
            nc.vector.tensor_copy(a8_sb, a_ps)

            # all-gather a: every core needs the full [128, 512] map.
            # These DMAs ride the gpsimd queue so they don't wait behind the
            # bulk xseg/weight transfers on the sync queue.
            ain = dram.tile([128, BSH], f8, name="ain")
            aout = dram.tile([NCORES, 128, BSH], f8, name="aout")
            nc.gpsimd.dma_start(out=ain, in_=a8_sb)
            nc.gpsimd.collective_compute(
                "AllGather", mybir.AluOpType.bypass,
                replica_groups=[list(range(NCORES))],
                ins=[ain.opt()],
                outs=[aout.rearrange("g p b -> (g p) b").opt()],
            )
            nc.gpsimd.dma_start(
                out=aT_sb, in_=aout.rearrange("g p b -> p g b"))
            aT = aT_sb.rearrange("p g b -> p (g b)")

            # keep the PE p-state ramped through the ~17us collective: the
            # loop's matmuls would otherwise restart at half clock for ~3us
            warm_ps = psum_g.tile([128, 4, WS], f32, tag="gfoB",
                                  name="warm_ps")
            wp = warm_ps.rearrange("p s w -> p (s w)")
            for i in range(int(os.environ.get("ENC_WARM", "123"))):
                nc.tensor.matmul(wp[:, 0:512], wih_sb[:, 0:128],
                                 wih_sb[:, 0:512], start=True, stop=True)

            # ---- recurrence ----
            yv = y_d.ap()
            base_pairs = [("A", 0), ("B", WS)]
            c_prev, h_prev, sp_cur = {}, {}, {}
            for P, bx in base_pairs:
                cP = st_pool.tile([128, 2, WS], f16, tag=f"c{P}",
                                  name=f"c_init{P}")
                nc.vector.memset(cP.rearrange("p a w -> p (a w)"), 0.0)
                hP = st_pool.tile([128, 2, WS], f8, tag=f"h{P}",
                                  name=f"h_init{P}")
                nc.vector.memset(hP.rearrange("p a w -> p (a w)"), 0.0)
                c_prev[P] = cP
                h_prev[P] = hP

            chunk_tiles = {}
            xh_tiles = {}
            gates_cur = {}

            def make_xh(P, bx, u):
                # one step ahead, on Pool: never on the DVE critical path
                xh = xh_pool.tile([128, WS], f16, tag=f"xh{P}",
                                  name=f"xh{P}_{u}")
                nc.gpsimd.tensor_mul(xh, xs_sb[:, u, bx:bx + WS],
                                     aT[:, bx:bx + WS])
                xh_tiles[(P, u)] = xh

            def phase_pre(P, bx, u):
                # two PSUM tiles per pair: {i,g} (head of the cell chain) and
                # {f,o}; sigmoid on {i,g} never waits for the {f,o} matmuls
                gig = psum_g.tile([128, 4, WS], f32, tag=f"gig{P}",
                                  name=f"gig{P}_{u}")
                gfo = psum_g.tile([128, 4, WS], f32, tag=f"gfo{P}",
                                  name=f"gfo{P}_{u}")
                for q in range(2):
                    nc.tensor.matmul(
                        gig[:, 2 * q:2 * q + 2, :]
                        .rearrange("p s w -> p (s w)"),
                        b8_sb, e8_sb[:, (2 * q) * WS:(2 * q + 2) * WS],
                        start=True, stop=False)
                    nc.tensor.matmul(
                        gfo[:, 2 * q:2 * q + 2, :]
                        .rearrange("p s w -> p (s w)"),
                        b8_sb, e8_sb[:, (4 + 2 * q) * WS:(6 + 2 * q) * WS],
                        start=True, stop=False)
                xh = xh_tiles.pop((P, u))
                for s in range(4):
                    nc.tensor.matmul(gig[:, s, :],
                                     wih_sb[:, s * 128:(s + 1) * 128],
                                     xh, start=False, stop=False)
                for s in range(4, 8):
                    nc.tensor.matmul(gfo[:, s - 4, :],
                                     wih_sb[:, s * 128:(s + 1) * 128],
                                     xh, start=False, stop=False)
                gates_cur[P] = (gig, gfo)

            def phase_h(P, bx, u):
                gig, gfo = gates_cur[P]
                hp = h_prev[P]
                wlist = [whh8_sb, whhd_sb] if WRES else [whh8_sb]
                for wsb in wlist:
                    for s in range(4):
                        nc.tensor.matmul(gig[:, s, :], wsb[:, :, s, :], hp,
                                         start=False,
                                         stop=(s == 3 and wsb is wlist[-1]),
                                         perf_mode=PM.DoubleRow)
                sp = sp_pool.tile([128, 8, WS], f16, tag=f"sp{P}",
                                  name=f"sp{P}_{u}")
                sp_cur[P] = sp
                nc.scalar.activation(
                    sp[:, 0:4, :].rearrange("p s w -> p (s w)"),
                    gig.rearrange("p s w -> p (s w)"), AF.Sigmoid)
                for wsb in wlist:
                    for s in range(4, 8):
                        nc.tensor.matmul(gfo[:, s - 4, :], wsb[:, :, s, :],
                                         hp, start=False,
                                         stop=(s == 7 and wsb is wlist[-1]),
                                         perf_mode=PM.DoubleRow)
                nc.scalar.activation(
                    sp[:, 4:8, :].rearrange("p s w -> p (s w)"),
                    gfo.rearrange("p s w -> p (s w)"), AF.Sigmoid)

            def cell_head(P, bx, u):
                # v = (sig(2zg)-0.5)*sig(zi) = tanh(zg)*sig(zi)/2, the c/2
                # contribution of the input gate; only needs the {i,g} half
                sp = sp_cur[P]
                spf = sp.rearrange("p s w -> p (s w)")
                v = tmp_pool.tile([128, 2 * WS], f16, tag=f"v{P}",
                                  name=f"v{P}_{u}")
                nc.vector.scalar_tensor_tensor(
                    out=v, in0=spf[:, 2 * WS:4 * WS], scalar=0.5,
                    in1=spf[:, 0:2 * WS], op0=OP.subtract, op1=OP.mult)
                return v

            def cell_tail(P, bx, u, v):
                # device carries c' = c/2: c'_new = sig_f*c' + v, so the cell
                # add is a 2x-mode tensor_tensor; tanh(c) = 2*sig(4c')-1
                sp = sp_cur[P]
                spf = sp.rearrange("p s w -> p (s w)")
                t1 = tmp_pool.tile([128, 2 * WS], f16, tag=f"t1{P}",
                                   name=f"t1{P}_{u}")
                nc.vector.tensor_mul(
                    t1, spf[:, 4 * WS:6 * WS],
                    c_prev[P].rearrange("p a w -> p (a w)"))
                cn = st_pool.tile([128, 2, WS], f16, tag=f"c{P}",
                                  name=f"c{P}_{u}")
                nc.vector.tensor_add(
                    cn.rearrange("p a w -> p (a w)"), v, t1)
                c_prev[P] = cn
                # tanh table shares a function set with sigmoid: no reloads
                th = tmp_pool.tile([128, 2, WS], f16, tag=f"th{P}",
                                   name=f"th{P}_{u}")
                nc.scalar.activation(
                    th.rearrange("p a w -> p (a w)"),
                    cn.rearrange("p a w -> p (a w)"), AF.Tanh, scale=2.0)
                h8 = st_pool.tile([128, 2, WS], f8, tag=f"h{P}",
                                  name=f"h{P}_{u}")
                nc.vector.tensor_mul(
                    h8.rearrange("p a w -> p (a w)"),
                    th.rearrange("p a w -> p (a w)"),
                    spf[:, 6 * WS:8 * WS])
                h_prev[P] = h8
                # fp16 output copy runs on Pool, off the critical path
                hsl = chunk_tiles[u // TCH][:, u % TCH, :, bx:bx + WS]
                nc.gpsimd.tensor_mul(hsl, th, sp[:, 6:8, :])

            pairs = list(base_pairs)
            for P, bx in pairs:
                make_xh(P, bx, 0)
            for u in range(NS):
                pairs = base_pairs if u % 2 == 0 else base_pairs[::-1]
                if u % TCH == 0:
                    chunk_tiles[u // TCH] = out_pool.tile(
                        [128, TCH, 2, BC], f16, tag="hout",
                        name=f"hout{u // TCH}")
                if u + 1 < NS:
                    for P, bx in pairs:
                        make_xh(P, bx, u + 1)
                for P, bx in pairs:
                    phase_pre(P, bx, u)
                for P, bx in pairs:
                    phase_h(P, bx, u)
                for P, bx in pairs:
                    vP = cell_head(P, bx, u)
                    cell_tail(P, bx, u, vP)
                if u in (0, 2, 4):
                    # the DVE seq reaches this point only once step u's cell
                    # ops have issued, so these bulk transfers cannot race
                    # ahead of the small attention/all-gather DMAs
                    load_xseg(u // 2 + 1, eng=nc.gpsimd)
                if u % TCH == TCH - 1 or u == NS - 1:
                    ci = u // TCH
                    n_t = (u % TCH) + 1
                    nc.sync.dma_start(
                        out=yv[:, ci * TCH * 2:ci * TCH * 2 + n_t * 2, :],
                        in_=chunk_tiles[ci][:, 0:n_t, :, :]
                        .rearrange("p t ht b -> p (t ht) b"))

    nc.compile()
    return nc


def _seg_offsets():
    return [s * SEGV for s in range(T_SHARD)]


def _prepare_in_maps(inputs):
    f8 = ml_dtypes.float8_e4m3
    f16 = np.float16
    x = np.asarray(inputs["x"], np.float32)
    attn_w = np.asarray(inputs["attn_w"], np.float32)
    W_ih = np.asarray(inputs["W_ih"], np.float32)
    W_hh = np.asarray(inputs["W_hh"], np.float32)
    b = (np.asarray(inputs["b_ih"], np.float32)
         + np.asarray(inputs["b_hh"], np.float32))

    wx = np.ascontiguousarray(attn_w[2 * H:])
    wxt = np.ascontiguousarray(wx.reshape(2, 128).T).astype(f8)

    # gate order (i,f,g,o); g rows x2 for tanh(z)=2sig(2z)-1; h is carried
    # at full scale (tanh table); slots permuted to (i,g | f,o).
    gate_scale = np.ones((4 * H, 1), np.float32)
    gate_scale[2 * H:3 * H] = 2.0
    W_ih = W_ih * gate_scale
    W_hh = W_hh * gate_scale
    b = b * gate_scale[:, 0]

    wih_re = np.ascontiguousarray(
        W_ih.T.reshape(D, 8, 128)[:, PERM, :].reshape(D, 8 * 128)).astype(f16)
    wt = np.ascontiguousarray(
        W_hh.T.reshape(2, 128, 8, 128)[:, :, PERM, :]
        .transpose(1, 0, 2, 3))                               # [128,2,8,128]
    whh8 = wt.astype(f8)
    whhd = (wt - whh8.astype(np.float32)).astype(f8)
    b8 = np.ascontiguousarray(b.reshape(8, 128)[PERM, :]).astype(f16)
    e8 = np.repeat(np.eye(8, dtype=np.float32), WS, axis=1).astype(f16)

    shared = {"wxt": wxt, "wih": wih_re, "whh8": whh8, "whhd": whhd,
              "b8dr": b8, "e8dr": e8}

    xf8 = x.astype(f8)
    xdtb = np.ascontiguousarray(xf8.transpose(1, 2, 0))   # [D, W, B] fp8
    offs = _seg_offsets()
    in_maps = []
    for c in range(NCORES):
        t0 = offs[c]
        m = dict(shared)
        m["xseg"] = np.ascontiguousarray(xdtb[:, t0:t0 + NS, :])
        xb = x[c * BSH:(c + 1) * BSH]                     # [64, D, W] f32
        m["xt8"] = np.ascontiguousarray(
            xb.transpose(2, 1, 0).reshape(2, 128, D, 4, BSH // 4)
            .transpose(0, 3, 1, 2, 4)).astype(f8)
        in_maps.append(m)
    return in_maps


def _make_runner(nc):
    import jax
    from jax.sharding import Mesh, PartitionSpec, NamedSharding
    from jax.experimental.shard_map import shard_map
    from concourse import mybir
    from concourse.bass2jax import (_bass_exec_p, install_neuronx_cc_hook,
                                    partition_id_tensor)

    install_neuronx_cc_hook()
    pname = nc.partition_id_tensor.name if nc.partition_id_tensor else None
    in_names, out_names, out_avals, zero_outs = [], [], [], []
    for alloc in nc.m.functions[0].allocations:
        if not isinstance(alloc, mybir.MemoryLocationSet):
            continue
        name = alloc.memorylocations[0].name
        if alloc.kind == "ExternalInput":
            if name != pname:
                in_names.append(name)
        elif alloc.kind == "ExternalOutput":
            shape = tuple(alloc.tensor_shape)
            dtype = mybir.dt.np(alloc.dtype)
            out_avals.append(jax.core.ShapedArray(shape, dtype))
            zero_outs.append(np.zeros(shape, dtype))
            out_names.append(name)
    n_params = len(in_names)
    all_names = in_names + out_names
    if pname is not None:
        all_names = all_names + [pname]

    def _body(*args):
        operands = list(args)
        if pname is not None:
            operands.append(partition_id_tensor())
        return tuple(_bass_exec_p.bind(
            *operands,
            out_avals=tuple(out_avals),
            in_names=tuple(all_names),
            out_names=tuple(out_names),
            lowering_input_output_aliases=(),
            sim_require_finite=True,
            sim_require_nnan=True,
            nc=nc,
        ))

    devices = jax.devices()[:NCORES]
    mesh = Mesh(np.asarray(devices), ("core",))
    nspec = (PartitionSpec("core"),)
    jitted = jax.jit(
        shard_map(_body, mesh=mesh,
                  in_specs=nspec * (n_params + len(out_names)),
                  out_specs=nspec * len(out_names),
                  check_rep=False),
        keep_unused=True)
    sharding = NamedSharding(mesh, PartitionSpec("core"))
    resident_zeros = [
        jax.device_put(
            np.zeros((NCORES * z.shape[0], *z.shape[1:]), z.dtype),
            sharding)
        for z in zero_outs
    ]
    return jitted, in_names, resident_zeros, sharding


def kernel(**inputs) -> np.ndarray:
    global LAST_EXEC_NS
    import jax

    if "prog" not in _CACHE:
        nc = _build_program()
        _CACHE["prog"] = _make_runner(nc)
    jitted, in_names, resident_zeros, sharding = _CACHE["prog"]

    from concurrent.futures import ThreadPoolExecutor

    in_maps = _prepare_in_maps(inputs)
    concat_in = [
        jax.device_put(
            np.concatenate([in_maps[c][n] for c in range(NCORES)], axis=0),
            sharding)
        for n in in_names
    ]
    try:
        outs = jitted(*concat_in, *resident_zeros)
        jax.block_until_ready(outs)
    except Exception:
        outs = jitted(*concat_in, *resident_zeros)
        jax.block_until_ready(outs)

    out = np.empty((B, W, H), np.float32)
    shards = sorted(outs[0].addressable_shards, key=lambda s: s.index[0])
    s_data = [sh.data for sh in shards]
    offs = _seg_offsets()

    def fetch_one(c):
        arr = np.asarray(s_data[c]).reshape(128, NS, 2, BC)
        u_lo = 0 if c == 0 else LWARM
        arr = arr[:, u_lo:].astype(np.float32)
        nt = NS - u_lo
        out[:, offs[c] + u_lo: offs[c] + u_lo + nt] = (
            arr.transpose(3, 1, 2, 0).reshape(BC, nt, H))

    with ThreadPoolExecutor(NCORES) as ex:
        list(ex.map(fetch_one, range(NCORES)))
    return out


# revision 45
# speedup vs baseline: 1.0008x; 1.0008x over previous
"""Trainium2 Bass kernel for nn_Encoder_55293408969294 — v3: 8-way time shard.

Why v3 beats v2 (334 us):
  - The wall is the per-step serial chain  h_t -> W_hh matmuls -> sigmoid ->
    cell -> tanh -> h_{t+1}  (~4.2 us in v2), which no amount of stream
    interleaving can pipeline away: step t+1 of a stream cannot start before
    its own step t finishes.  Total time ~= NS * chain.  v3 therefore cuts NS:
    8 time segments (one per core, FULL batch 512 per core) with an L=8-step
    zero-state warmup, NS = (256+7*8)/8 = 39 steps (vs 70), and instead makes
    each step WIDER (2 batch pairs of 256) so the engines stay busy inside the
    longer-but-fewer chains.
  - h is carried in fp8-e4m3 and the h-side matmuls use DoubleRow perf mode
    (fp8, 2 k-tiles of 128 = the two hidden halves in one 0.5-cycle/row
    matmul).  Weight quantization noise is killed by a two-term expansion
    W_hh ~ fp8(W) + fp8(W - fp8(W)): two DoubleRow matmuls at half cost each,
    ~fp16 weight fidelity (1.2e-2 end-to-end vs 1.5e-2 single-term; h-fp8
    itself adds ~8e-3 over the 5e-3 numpy baseline; budget 2e-2).
  - All 8 gate slots get ONE sigmoid ACT call per pair ([128,2048]; ACT has a
    185ns fixed access overhead per call) using tanh(z)=2*sig(2z)-1 with the
    2x folded into host-scaled weights; tanh(c) is one sig(2c) call.
  - The per-gate bias enters PSUM via a single DoubleRow matmul from an
    8-partition one-hot (out free 2048 at 0.2 ns/row), not 2 fp16 matmuls.
  - DVE keeps only the latency-critical cell chain (v, t1, cn, h8); the
    off-chain products (xh = a*x for step u+1, and the fp16 output copy
    y = h') run on the otherwise-idle GPSIMD/Pool engine.
  - Attention (softmax over drives of e_x = x . w_x, constant over t) is
    batch-sharded: each core reduces its OWN 64 batch rows over the full
    window from a [t,d,b] fp8 transpose (2.1 MB, not 8.4), then an AllGather
    collective (64 KB) replicates a to all cores.
"""

import os
import numpy as np
import ml_dtypes  # noqa: F401

B, D, W, H = 512, 128, 256, 256
NCORES = 8

T_SHARD = 8
LWARM = int(os.environ.get("ENC_L", "8"))
NS = (W + (T_SHARD - 1) * LWARM) // T_SHARD   # 39 local steps per core
SEGV = NS - LWARM                             # valid steps, segments >= 1
BC = B                                        # full batch per core
WS = BC // 2                                  # pair width (256)
BSH = B // NCORES                             # attention batch shard (64)
TCH = int(os.environ.get("ENC_TCH", "1"))
WRES = os.environ.get("ENC_WRES", "1") == "1"

# slot order (i0,i1,g0,g1,f0,f1,o0,o1): the {i,g} half feeds v (the head of
# the cell chain) from its own PSUM tile; {f,o} matmuls+sigmoid overlap it.
PERM = [0, 1, 4, 5, 2, 3, 6, 7]   # slot -> original (i,f,g,o) block

_CACHE = {}
LAST_EXEC_NS = None


def _build_program(*_ignored):
    import concourse.bacc as bacc
    import concourse.bass as bass  # noqa: F401
    import concourse.mybir as mybir
    import concourse.tile as tile
    from concourse.masks import make_identity
    from contextlib import ExitStack

    f32 = mybir.dt.float32
    f16 = mybir.dt.float16
    f8 = mybir.dt.float8e4
    AF = mybir.ActivationFunctionType
    OP = mybir.AluOpType
    PM = mybir.MatmulPerfMode

    nc = bacc.Bacc("TRN2", target_bir_lowering=False, debug=False)

    xs_d = nc.dram_tensor("xseg", [D, NS, BC], f8, kind="ExternalInput")
    xt_d = nc.dram_tensor("xt8", [2, 4, 128, D, BSH // 4], f8,
                          kind="ExternalInput")
    wxt_d = nc.dram_tensor("wxt", [128, 2], f8, kind="ExternalInput")
    wih_d = nc.dram_tensor("wih", [D, 8 * 128], f16, kind="ExternalInput")
    whh8_d = nc.dram_tensor("whh8", [128, 2, 8, 128], f8, kind="ExternalInput")
    whhd_d = nc.dram_tensor("whhd", [128, 2, 8, 128], f8, kind="ExternalInput")
    b8_d = nc.dram_tensor("b8dr", [8, 128], f16, kind="ExternalInput")
    e8_d = nc.dram_tensor("e8dr", [8, 8 * WS], f16, kind="ExternalInput")
    # out: y[p, u*2+ht, b] = h_u[ht*128+p, b]
    y_d = nc.dram_tensor("y", [128, NS * 2, BC], f16, kind="ExternalOutput")

    with tile.TileContext(nc) as tc:
        with ExitStack() as ctx:
            singles = ctx.enter_context(tc.tile_pool(name="singles", bufs=1))
            psum_g = ctx.enter_context(
                tc.tile_pool(name="pg", bufs=1, space="PSUM"))
            dram = ctx.enter_context(
                tc.tile_pool(name="dram", bufs=1, space="DRAM"))
            xh_pool = ctx.enter_context(tc.tile_pool(name="xhp", bufs=3))
            sp_pool = ctx.enter_context(tc.tile_pool(name="spp", bufs=2))
            tmp_pool = ctx.enter_context(tc.tile_pool(name="tmpp", bufs=2))
            st_pool = ctx.enter_context(tc.tile_pool(name="stp", bufs=2))
            out_pool = ctx.enter_context(tc.tile_pool(name="outp", bufs=3))

            xs_sb = singles.tile([D, NS, BC], f8, name="xs_sb")
            xt_sb = singles.tile([128, 2, 4, D, BSH // 4], f8, name="xt_sb")
            wxt_sb = singles.tile([128, 2], f8, name="wxt_sb")
            wih_sb = singles.tile([128, 8 * 128], f16, name="wih_sb")
            whh8_sb = singles.tile([128, 2, 8, 128], f8, name="whh8_sb")
            whhd_sb = singles.tile([128, 2, 8, 128], f8, name="whhd_sb")
            b8_sb = singles.tile([8, 128], f16, name="b8_sb")
            e8_sb = singles.tile([8, 8 * WS], f16, name="e8_sb")
            id_sb = singles.tile([128, 128], f32, name="id_sb")
            exT = singles.tile([128, BSH], f32, name="exT")
            sg_sb = singles.tile([BSH, 128], f32, name="sg_sb")
            den_sb = singles.tile([BSH, 128], f32, name="den_sb")
            num_sb = singles.tile([BSH, 128], f32, name="num_sb")
            ssum = singles.tile([BSH, 1], f32, name="ssum")
            rr = singles.tile([BSH, 1], f32, name="rr")
            ab_sb = singles.tile([BSH, 128], f32, name="ab_sb")
            a8_sb = singles.tile([128, BSH], f8, name="a8_sb")
            aT_sb = singles.tile([128, NCORES, BSH], f8, name="aT_sb")

            # DMA priority: wxt + the attention transpose feed the collective
            # that gates the whole recurrence; weights follow; xseg last.
            nc.sync.dma_start(out=wxt_sb, in_=wxt_d.ap())
            make_identity(nc, id_sb)
            # b-chunk-major host layout: every chunk is contiguous on both
            # sides, so the attention matmuls pipeline under the transfer
            xtr = xt_d.ap().rearrange("tt bq tp d b -> tp tt bq d b")
            BQ = BSH // 4
            for tt in range(2):
                nc.sync.dma_start(
                    out=xt_sb[:, tt], in_=xtr[:, tt])
            nc.sync.dma_start(out=wih_sb, in_=wih_d.ap())
            nc.sync.dma_start(out=whh8_sb, in_=whh8_d.ap())
            if WRES:
                nc.sync.dma_start(out=whhd_sb, in_=whhd_d.ap())
            nc.sync.dma_start(out=b8_sb, in_=b8_d.ap())
            nc.sync.dma_start(out=e8_sb, in_=e8_d.ap())
            # only the first xseg chunk is loaded up front: the rest would
            # queue ahead of the tiny all-gather bounce DMA on the serialized
            # DMA engines and delay the collective; they stream in during the
            # first loop steps instead (chunk k is consumed from step ~10k).
            XCH = (NS + 3) // 4

            def load_xseg(tk, eng=None):
                lo = tk * XCH
                hi = min(lo + XCH, NS)
                if lo < hi:
                    (eng or nc.sync).dma_start(
                        out=xs_sb[:, lo:hi, :], in_=xs_d.ap()[:, lo:hi, :])

            load_xseg(0)

            # ---- attention (own 64 batch rows over the full window) ----
            gA = psum_g.tile([128, 4, WS], f32, tag="gigA", name="gA_att")
            gB = psum_g.tile([128, 4, WS], f32, tag="gigB", name="gB_att")
            gC = psum_g.tile([128, 4, WS], f32, tag="gfoA", name="gC_att")
            e_ps = gA.rearrange("p s w -> p (s w)")[:, 0:BSH]
            eb_ps = gB[0:BSH].rearrange("p s w -> p (s w)")[:, 0:128]
            for tt in range(2):
                for b in range(BSH):
                    nc.tensor.matmul(
                        e_ps[:, b:b + 1],
                        xt_sb[:, tt, b // BQ, :, b % BQ],
                        wxt_sb[:, tt:tt + 1],
                        start=(tt == 0), stop=(tt == 1))
            nc.vector.tensor_copy(exT, e_ps)
            nc.tensor.transpose(eb_ps, exT, id_sb)
            # exp(z) = sig(z)/(1-sig(z)) keeps everything on the Sigmoid table
            nc.scalar.activation(sg_sb, eb_ps, AF.Sigmoid)
            nc.vector.tensor_scalar(
                out=den_sb, in0=sg_sb, scalar1=-1.0, scalar2=1.0,
                op0=OP.mult, op1=OP.add)
            nc.vector.reciprocal(den_sb, den_sb)
            nc.vector.scalar_tensor_tensor(
                out=num_sb, in0=sg_sb, scalar=1.0, in1=den_sb,
                op0=OP.mult, op1=OP.mult, accum_out=ssum)
            nc.vector.reciprocal(rr, ssum)
            nc.vector.tensor_scalar_mul(ab_sb, num_sb, rr)
            a_ps = gC.rearrange("p s w -> p (s w)")[:, 0:BSH]
            nc.tensor.transpose(a_ps, ab_sb, id_sb[0:BSH, 0:BSH])
            nc.vector.tensor_copy(a8_sb, a_ps)

            # all-gather a: every core needs the full [128, 512] map.
            # These DMAs ride the gpsimd queue so they don't wait behind the
            # bulk xseg/weight transfers on the sync queue.
            ain = dram.tile([128, BSH], f8, name="ain")
            aout = dram.tile([NCORES, 128, BSH], f8, name="aout")
            nc.scalar.dma_start(out=ain, in_=a8_sb)
            nc.gpsimd.collective_compute(
                "AllGather", mybir.AluOpType.bypass,
                replica_groups=[list(range(NCORES))],
                ins=[ain.opt()],
                outs=[aout.rearrange("g p b -> (g p) b").opt()],
            )
            nc.scalar.dma_start(
                out=aT_sb, in_=aout.rearrange("g p b -> p g b"))
            aT = aT_sb.rearrange("p g b -> p (g b)")

            # keep the PE p-state ramped through the ~17us collective: the
            # loop's matmuls would otherwise restart at half clock for ~3us
            warm_ps = psum_g.tile([128, 4, WS], f32, tag="gfoB",
                                  name="warm_ps")
            wp = warm_ps.rearrange("p s w -> p (s w)")
            for i in range(int(os.environ.get("ENC_WARM", "123"))):
                nc.tensor.matmul(wp[:, 0:512], wih_sb[:, 0:128],
                                 wih_sb[:, 0:512], start=True, stop=True)

            # ---- recurrence ----
            yv = y_d.ap()
            base_pairs = [("A", 0), ("B", WS)]
            c_prev, h_prev, sp_cur = {}, {}, {}
            for P, bx in base_pairs:
                cP = st_pool.tile([128, 2, WS], f16, tag=f"c{P}",
                                  name=f"c_init{P}")
                nc.vector.memset(cP.rearrange("p a w -> p (a w)"), 0.0)
                hP = st_pool.tile([128, 2, WS], f8, tag=f"h{P}",
                                  name=f"h_init{P}")
                nc.vector.memset(hP.rearrange("p a w -> p (a w)"), 0.0)
                c_prev[P] = cP
                h_prev[P] = hP

            chunk_tiles = {}
            xh_tiles = {}
            gates_cur = {}

            def make_xh(P, bx, u):
                # one step ahead, on Pool: never on the DVE critical path
                xh = xh_pool.tile([128, WS], f16, tag=f"xh{P}",
                                  name=f"xh{P}_{u}")
                nc.gpsimd.tensor_mul(xh, xs_sb[:, u, bx:bx + WS],
                                     aT[:, bx:bx + WS])
                xh_tiles[(P, u)] = xh

            def phase_pre(P, bx, u):
                # two PSUM tiles per pair: {i,g} (head of the cell chain) and
                # {f,o}; sigmoid on {i,g} never waits for the {f,o} matmuls
                gig = psum_g.tile([128, 4, WS], f32, tag=f"gig{P}",
                                  name=f"gig{P}_{u}")
                gfo = psum_g.tile([128, 4, WS], f32, tag=f"gfo{P}",
                                  name=f"gfo{P}_{u}")
                for q in range(2):
                    nc.tensor.matmul(
                        gig[:, 2 * q:2 * q + 2, :]
                        .rearrange("p s w -> p (s w)"),
                        b8_sb, e8_sb[:, (2 * q) * WS:(2 * q + 2) * WS],
                        start=True, stop=False)
                    nc.tensor.matmul(
                        gfo[:, 2 * q:2 * q + 2, :]
                        .rearrange("p s w -> p (s w)"),
                        b8_sb, e8_sb[:, (4 + 2 * q) * WS:(6 + 2 * q) * WS],
                        start=True, stop=False)
                xh = xh_tiles.pop((P, u))
                for s in range(4):
                    nc.tensor.matmul(gig[:, s, :],
                                     wih_sb[:, s * 128:(s + 1) * 128],
                                     xh, start=False, stop=False)
                for s in range(4, 8):
                    nc.tensor.matmul(gfo[:, s - 4, :],
                                     wih_sb[:, s * 128:(s + 1) * 128],
                                     xh, start=False, stop=False)
                gates_cur[P] = (gig, gfo)

            def phase_h(P, bx, u):
                gig, gfo = gates_cur[P]
                hp = h_prev[P]
                wlist = [whh8_sb, whhd_sb] if WRES else [whh8_sb]
                for wsb in wlist:
                    for s in range(4):
                        nc.tensor.matmul(gig[:, s, :], wsb[:, :, s, :], hp,
                                         start=False,
                                         stop=(s == 3 and wsb is wlist[-1]),
                                         perf_mode=PM.DoubleRow)
                sp = sp_pool.tile([128, 8, WS], f16, tag=f"sp{P}",
                                  name=f"sp{P}_{u}")
                sp_cur[P] = sp
                nc.scalar.activation(
                    sp[:, 0:4, :].rearrange("p s w -> p (s w)"),
                    gig.rearrange("p s w -> p (s w)"), AF.Sigmoid)
                for wsb in wlist:
                    for s in range(4, 8):
                        nc.tensor.matmul(gfo[:, s - 4, :], wsb[:, :, s, :],
                                         hp, start=False,
                                         stop=(s == 7 and wsb is wlist[-1]),
                                         perf_mode=PM.DoubleRow)
                nc.scalar.activation(
                    sp[:, 4:8, :].rearrange("p s w -> p (s w)"),
                    gfo.rearrange("p s w -> p (s w)"), AF.Sigmoid)

            def cell_head(P, bx, u):
                # v = (sig(2zg)-0.5)*sig(zi) = tanh(zg)*sig(zi)/2, the c/2
                # contribution of the input gate; only needs the {i,g} half
                sp = sp_cur[P]
                spf = sp.rearrange("p s w -> p (s w)")
                v = tmp_pool.tile([128, 2 * WS], f16, tag=f"v{P}",
                                  name=f"v{P}_{u}")
                nc.vector.scalar_tensor_tensor(
                    out=v, in0=spf[:, 2 * WS:4 * WS], scalar=0.5,
                    in1=spf[:, 0:2 * WS], op0=OP.subtract, op1=OP.mult)
                return v

            def cell_tail(P, bx, u, v):
                # device carries c' = c/2: c'_new = sig_f*c' + v, so the cell
                # add is a 2x-mode tensor_tensor; tanh(c) = 2*sig(4c')-1
                sp = sp_cur[P]
                spf = sp.rearrange("p s w -> p (s w)")
                t1 = tmp_pool.tile([128, 2 * WS], f16, tag=f"t1{P}",
                                   name=f"t1{P}_{u}")
                nc.vector.tensor_mul(
                    t1, spf[:, 4 * WS:6 * WS],
                    c_prev[P].rearrange("p a w -> p (a w)"))
                cn = st_pool.tile([128, 2, WS], f16, tag=f"c{P}",
                                  name=f"c{P}_{u}")
                nc.vector.tensor_add(
                    cn.rearrange("p a w -> p (a w)"), v, t1)
                c_prev[P] = cn
                # tanh table shares a function set with sigmoid: no reloads
                th = tmp_pool.tile([128, 2, WS], f16, tag=f"th{P}",
                                   name=f"th{P}_{u}")
                nc.scalar.activation(
                    th.rearrange("p a w -> p (a w)"),
                    cn.rearrange("p a w -> p (a w)"), AF.Tanh, scale=2.0)
                h8 = st_pool.tile([128, 2, WS], f8, tag=f"h{P}",
                                  name=f"h{P}_{u}")
                nc.vector.tensor_mul(
                    h8.rearrange("p a w -> p (a w)"),
                    th.rearrange("p a w -> p (a w)"),
                    spf[:, 6 * WS:8 * WS])
                h_prev[P] = h8
                # fp16 output copy runs on Pool, off the critical path
                hsl = chunk_tiles[u // TCH][:, u % TCH, :, bx:bx + WS]
                nc.gpsimd.tensor_mul(hsl, th, sp[:, 6:8, :])

            pairs = list(base_pairs)
            for P, bx in pairs:
                make_xh(P, bx, 0)
            for u in range(NS):
                pairs = base_pairs if u % 2 == 0 else base_pairs[::-1]
                if u % TCH == 0:
                    chunk_tiles[u // TCH] = out_pool.tile(
                        [128, TCH, 2, BC], f16, tag="hout",
                        name=f"hout{u // TCH}")
                if u + 1 < NS:
                    for P, bx in pairs:
                        make_xh(P, bx, u + 1)
                for P, bx in pairs:
                    phase_pre(P, bx, u)
                for P, bx in pairs:
                    phase_h(P, bx, u)
                for P, bx in pairs:
                    vP = cell_head(P, bx, u)
                    cell_tail(P, bx, u, vP)
                if u in (0, 2, 4):
                    # the DVE seq reaches this point only once step u's cell
                    # ops have issued, so these bulk transfers cannot race
                    # ahead of the small attention/all-gather DMAs
                    load_xseg(u // 2 + 1, eng=nc.gpsimd)
                if u % TCH == TCH - 1 or u == NS - 1:
                    ci = u // TCH
                    n_t = (u % TCH) + 1
                    nc.sync.dma_start(
                        out=yv[:, ci * TCH * 2:ci * TCH * 2 + n_t * 2, :],
                        in_=chunk_tiles[ci][:, 0:n_t, :, :]
                        .rearrange("p t ht b -> p (t ht) b"))

    nc.compile()
    return nc


def _seg_offsets():
    return [s * SEGV for s in range(T_SHARD)]


def _prepare_in_maps(inputs):
    f8 = ml_dtypes.float8_e4m3
    f16 = np.float16
    x = np.asarray(inputs["x"], np.float32)
    attn_w = np.asarray(inputs["attn_w"], np.float32)
    W_ih = np.asarray(inputs["W_ih"], np.float32)
    W_hh = np.asarray(inputs["W_hh"], np.float32)
    b = (np.asarray(inputs["b_ih"], np.float32)
         + np.asarray(inputs["b_hh"], np.float32))

    wx = np.ascontiguousarray(attn_w[2 * H:])
    wxt = np.ascontiguousarray(wx.reshape(2, 128).T).astype(f8)

    # gate order (i,f,g,o); g rows x2 for tanh(z)=2sig(2z)-1; h is carried
    # at full scale (tanh table); slots permuted to (i,g | f,o).
    gate_scale = np.ones((4 * H, 1), np.float32)
    gate_scale[2 * H:3 * H] = 2.0
    W_ih = W_ih * gate_scale
    W_hh = W_hh * gate_scale
    b = b * gate_scale[:, 0]

    wih_re = np.ascontiguousarray(
        W_ih.T.reshape(D, 8, 128)[:, PERM, :].reshape(D, 8 * 128)).astype(f16)
    wt = np.ascontiguousarray(
        W_hh.T.reshape(2, 128, 8, 128)[:, :, PERM, :]
        .transpose(1, 0, 2, 3))                               # [128,2,8,128]
    whh8 = wt.astype(f8)
    whhd = (wt - whh8.astype(np.float32)).astype(f8)
    b8 = np.ascontiguousarray(b.reshape(8, 128)[PERM, :]).astype(f16)
    e8 = np.repeat(np.eye(8, dtype=np.float32), WS, axis=1).astype(f16)

    shared = {"wxt": wxt, "wih": wih_re, "whh8": whh8, "whhd": whhd,
              "b8dr": b8, "e8dr": e8}

    xf8 = x.astype(f8)
    xdtb = np.ascontiguousarray(xf8.transpose(1, 2, 0))   # [D, W, B] fp8
    offs = _seg_offsets()
    in_maps = []
    for c in range(NCORES):
        t0 = offs[c]
        m = dict(shared)
        m["xseg"] = np.ascontiguousarray(xdtb[:, t0:t0 + NS, :])
        xb = x[c * BSH:(c + 1) * BSH]                     # [64, D, W] f32
        m["xt8"] = np.ascontiguousarray(
            xb.transpose(2, 1, 0).reshape(2, 128, D, 4, BSH // 4)
            .transpose(0, 3, 1, 2, 4)).astype(f8)
        in_maps.append(m)
    return in_maps


def _make_runner(nc):
    import jax
    from jax.sharding import Mesh, PartitionSpec, NamedSharding
    from jax.experimental.shard_map import shard_map
    from concourse import mybir
    from concourse.bass2jax import (_bass_exec_p, install_neuronx_cc_hook,
                                    partition_id_tensor)

    install_neuronx_cc_hook()
    pname = nc.partition_id_tensor.name if nc.partition_id_tensor else None
    in_names, out_names, out_avals, zero_outs = [], [], [], []
    for alloc in nc.m.functions[0].allocations:
        if not isinstance(alloc, mybir.MemoryLocationSet):
            continue
        name = alloc.memorylocations[0].name
        if alloc.kind == "ExternalInput":
            if name != pname:
                in_names.append(name)
        elif alloc.kind == "ExternalOutput":
            shape = tuple(alloc.tensor_shape)
            dtype = mybir.dt.np(alloc.dtype)
            out_avals.append(jax.core.ShapedArray(shape, dtype))
            zero_outs.append(np.zeros(shape, dtype))
            out_names.append(name)
    n_params = len(in_names)
    all_names = in_names + out_names
    if pname is not None:
        all_names = all_names + [pname]

    def _body(*args):
        operands = list(args)
        if pname is not None:
            operands.append(partition_id_tensor())
        return tuple(_bass_exec_p.bind(
            *operands,
            out_avals=tuple(out_avals),
            in_names=tuple(all_names),
            out_names=tuple(out_names),
            lowering_input_output_aliases=(),
            sim_require_finite=True,
            sim_require_nnan=True,
            nc=nc,
        ))

    devices = jax.devices()[:NCORES]
    mesh = Mesh(np.asarray(devices), ("core",))
    nspec = (PartitionSpec("core"),)
    jitted = jax.jit(
        shard_map(_body, mesh=mesh,
                  in_specs=nspec * (n_params + len(out_names)),
                  out_specs=nspec * len(out_names),
                  check_rep=False),
        keep_unused=True)
    sharding = NamedSharding(mesh, PartitionSpec("core"))
    resident_zeros = [
        jax.device_put(
            np.zeros((NCORES * z.shape[0], *z.shape[1:]), z.dtype),
            sharding)
        for z in zero_outs
    ]
    return jitted, in_names, resident_zeros, sharding


def kernel(**inputs) -> np.ndarray:
    global LAST_EXEC_NS
    import jax

    if "prog" not in _CACHE:
        nc = _build_program()
        _CACHE["prog"] = _make_runner(nc)
    jitted, in_names, resident_zeros, sharding = _CACHE["prog"]

    from concurrent.futures import ThreadPoolExecutor

    in_maps = _prepare_in_maps(inputs)
    concat_in = [
        jax.device_put(
            np.concatenate([in_maps[c][n] for c in range(NCORES)], axis=0),
            sharding)
        for n in in_names
    ]
    try:
        outs = jitted(*concat_in, *resident_zeros)
        jax.block_until_ready(outs)
    except Exception:
        outs = jitted(*concat_in, *resident_zeros)
        jax.block_until_ready(outs)

    out = np.empty((B, W, H), np.float32)
    shards = sorted(outs[0].addressable_shards, key=lambda s: s.index[0])
    s_data = [sh.data for sh in shards]
    offs = _seg_offsets()

    def fetch_one(c):
        arr = np.asarray(s_data[c]).reshape(128, NS, 2, BC)
        u_lo = 0 if c == 0 else LWARM
        arr = arr[:, u_lo:].astype(np.float32)
        nt = NS - u_lo
        out[:, offs[c] + u_lo: offs[c] + u_lo + nt] = (
            arr.transpose(3, 1, 2, 0).reshape(BC, nt, H))

    with ThreadPoolExecutor(NCORES) as ex:
        list(ex.map(fetch_one, range(NCORES)))
    return out
